# revision 1
# baseline (speedup 1.0000x reference)
"""OHEM cross-entropy loss kernel for Trainium2 (8 NeuronCores, Bass/Tile).

Math (matches reference.py):
    logp   = log_softmax(seg_logit, axis=1)          # [B,C,H,W], C=19
    x_l    = logp at label (ignore 255 -> class 0)
    prob   = exp(x_l)
    thr    = max(sort(prob.flatten())[MIN_KEPT*B], 0.7)
    loss   = mean(-x_l * (prob < thr))

Device strategy (data-parallel over B across 8 cores, one image per core):
    For each pixel p:  t = x_raw[label] - log(sum_c exp(x_raw[c]))  (= logp at label)
    w = 1[t < log(0.7)]   (valid when count(prob<0.7) > MIN_KEPT*B, which the
                           host verifies from the returned counts; otherwise a
                           host fallback computes the exact quantile path)
    Per-core partial sums of (t - log .7)*w (via min(u,0)) and of w are
    returned as [128, 16] partials; host combines:
        sum(-t*w) = -(sum_min + log(.7)*count)

    On-chip per 128x512-pixel chunk:
      - one fat DMA loads [128, 19, 512] f32 logits (class-major)
      - ACT: 19x exp -> bf16 expbuf; pairwise bulk adds (DVE 2x bf16) -> sumexp
      - label gather: 18 in-place copy_predicated mux-tree merges keyed on
        host-provided label bit-planes -> x_l in slot 0
      - ACT: lse = Ln(sumexp); DVE: u = (x_l - log.7) - lse;
        tensor_scalar accum_out reductions of min(u,0) and 1[u<0]
"""

import numpy as np

B = 8
C = 19
H, W = 512, 1024
HW = H * W            # 524288 pixels per image/core
P = 128               # SBUF partitions
FREE = HW // P        # 4096 pixels per partition
F = 512               # chunk free size
NCHUNK = FREE // F    # 8
NBITS = 5             # ceil(log2(19))
C0 = float(np.log(np.float32(0.7)))
MIN_KEPT = 100000
IGNORE_INDEX = 255
N_TOTAL = B * HW

_CACHE = {}


def _build_nc():
    import concourse.bacc as bacc
    import concourse.mybir as mybir
    import concourse.tile as tile

    fp32 = mybir.dt.float32
    bf16 = mybir.dt.bfloat16
    u8 = mybir.dt.uint8

    # Bacc (not plain Bass): its compile pass splits multi-sem sync waits,
    # which the mux-tree copy_predicated instructions need.
    nc = bacc.Bacc()
    logit = nc.dram_tensor("logit", [C, HW], fp32, kind="ExternalInput")
    bits = nc.dram_tensor("bits", [NBITS, P, FREE], u8, kind="ExternalInput")
    acc = nc.dram_tensor("acc", [P, 2 * NCHUNK], fp32, kind="ExternalOutput")

    # [C, (P FREE)] -> [P, C, FREE] view for chunked class-major loads
    logit_v = logit[:, :].rearrange("c (p f) -> p c f", p=P)

    with tile.TileContext(nc) as tc:
        with (
            tc.tile_pool(name="lb", bufs=2) as lb_pool,
            tc.tile_pool(name="eb", bufs=1) as eb_pool,
            tc.tile_pool(name="bits", bufs=1) as bits_pool,
            tc.tile_pool(name="pix", bufs=2) as pix_pool,
            tc.tile_pool(name="accp", bufs=1) as acc_pool,
        ):
            acc_t = acc_pool.tile([P, 2 * NCHUNK], fp32)
            bits_t = bits_pool.tile([P, NBITS, FREE], u8)
            # all 5 bit-planes in one DMA: [NBITS, P, FREE] -> [P, NBITS, FREE]
            nc.sync.dma_start(
                out=bits_t[:], in_=bits[:, :, :].rearrange("k p f -> p k f")
            )

            for j in range(NCHUNK):
                lb = lb_pool.tile([P, C, F], fp32, tag="lb")
                nc.sync.dma_start(out=lb[:], in_=logit_v[:, :, j * F : (j + 1) * F])

                eb = eb_pool.tile([P, C, F], bf16, tag="eb")
                for c in range(C):
                    nc.scalar.activation(
                        out=eb[:, c, :],
                        in_=lb[:, c, :],
                        func=mybir.ActivationFunctionType.Exp,
                    )

                # sumexp: pairwise bulk adds in bf16 (2x mode), final add in f32
                # tree: [0:9]+=[9:18]; [0:4]+=[4:8]; [8]+=[18]; [0:2]+=[2:4];
                #       [0]+=[1]; sumexp = [0]+[8] (f32 out)
                nc.vector.tensor_tensor(
                    out=eb[:, 0:9, :], in0=eb[:, 0:9, :], in1=eb[:, 9:18, :],
                    op=mybir.AluOpType.add,
                )
                nc.vector.tensor_tensor(
                    out=eb[:, 0:4, :], in0=eb[:, 0:4, :], in1=eb[:, 4:8, :],
                    op=mybir.AluOpType.add,
                )
                nc.vector.tensor_tensor(
                    out=eb[:, 8, :], in0=eb[:, 8, :], in1=eb[:, 18, :],
                    op=mybir.AluOpType.add,
                )
                nc.vector.tensor_tensor(
                    out=eb[:, 0:2, :], in0=eb[:, 0:2, :], in1=eb[:, 2:4, :],
                    op=mybir.AluOpType.add,
                )
                nc.vector.tensor_tensor(
                    out=eb[:, 0, :], in0=eb[:, 0, :], in1=eb[:, 1, :],
                    op=mybir.AluOpType.add,
                )
                sumexp = pix_pool.tile([P, F], fp32, tag="sumexp")
                nc.vector.tensor_tensor(
                    out=sumexp[:], in0=eb[:, 0, :], in1=eb[:, 8, :],
                    op=mybir.AluOpType.add,
                )

                lse = pix_pool.tile([P, F], fp32, tag="lse")
                nc.scalar.activation(
                    out=lse[:], in_=sumexp[:], func=mybir.ActivationFunctionType.Ln
                )

                # label mux-tree gather, in place on lb; merge (a, b, bit):
                # lb[:,a,:] <- lb[:,b,:] where bit-plane set
                merges = [
                    *[(2 * i, 2 * i + 1, 0) for i in range(9)],     # bit 0
                    (0, 2, 1), (4, 6, 1), (8, 10, 1), (12, 14, 1), (16, 18, 1),
                    (0, 4, 2), (8, 12, 2),                          # bit 2
                    (0, 8, 3),                                      # bit 3
                    (0, 16, 4),                                     # bit 4
                ]
                bslice = bits_t[:, :, j * F : (j + 1) * F]
                for a, b, k in merges:
                    nc.vector.copy_predicated(
                        out=lb[:, a, :], mask=bslice[:, k, :], data=lb[:, b, :]
                    )

                # u = (x_l - log0.7) - lse; partials: sum(min(u,0)), count(u<0)
                u = pix_pool.tile([P, F], fp32, tag="u")
                nc.vector.scalar_tensor_tensor(
                    out=u[:], in0=lb[:, 0, :], scalar=C0, in1=lse[:],
                    op0=mybir.AluOpType.subtract, op1=mybir.AluOpType.subtract,
                )
                # with accum_out, op1 is the reduce op: accum = reduce(out, op1)
                scr = pix_pool.tile([P, F], fp32, tag="scr")
                nc.vector.tensor_scalar(
                    out=scr[:], in0=u[:], scalar1=0.0, scalar2=None,
                    op0=mybir.AluOpType.min, op1=mybir.AluOpType.add,
                    accum_out=acc_t[:, j : j + 1],
                )
                scr2 = pix_pool.tile([P, F], fp32, tag="scr2")
                nc.vector.tensor_scalar(
                    out=scr2[:], in0=u[:], scalar1=0.0, scalar2=None,
                    op0=mybir.AluOpType.is_lt, op1=mybir.AluOpType.add,
                    accum_out=acc_t[:, NCHUNK + j : NCHUNK + j + 1],
                )

            nc.sync.dma_start(out=acc[:, :], in_=acc_t[:])
    nc.finalize()  # Bacc: runs compile() (reg alloc, sync-wait splitting)
    return nc


def _host_fallback(seg_logit, seg_label):
    """Exact numpy replication of the reference (quantile path included)."""
    x = np.asarray(seg_logit, dtype=np.float32)
    lbl = np.asarray(seg_label)
    Bn, Cn = x.shape[0], x.shape[1]
    xf = x.reshape(Bn, Cn, -1)
    m = xf.max(axis=1, keepdims=True)
    e = np.exp(xf - m)
    lse = np.log(e.sum(axis=1, keepdims=True)) + m
    logp = xf - lse
    l2 = np.where(lbl == IGNORE_INDEX, 0, lbl).reshape(Bn, 1, -1).astype(np.int64)
    lp_at = np.take_along_axis(logp, l2, axis=1)[:, 0]
    prob = np.exp(lp_at)
    sortp = np.sort(prob.reshape(-1))
    idx = min(MIN_KEPT * Bn, sortp.shape[0] - 1)
    thr = max(float(sortp[idx]), np.float32(0.7))
    wgt = (prob < thr).astype(np.float32)
    return np.float32((-lp_at * wgt).mean())


def kernel(seg_logit, seg_label):
    from concourse import bass_utils

    x = np.ascontiguousarray(np.asarray(seg_logit, dtype=np.float32)).reshape(
        B, C, HW
    )
    lbl = np.asarray(seg_label)
    lbl = np.where(lbl == IGNORE_INDEX, 0, lbl).astype(np.uint8).reshape(B, P, FREE)
    # 5 bit-planes per core: [NBITS, P, FREE] uint8
    bits = np.stack(
        [((lbl >> k) & 1).astype(np.uint8) for k in range(NBITS)], axis=1
    )  # [B, NBITS, P, FREE]

    if "nc" not in _CACHE:
        _CACHE["nc"] = _build_nc()
    nc = _CACHE["nc"]

    in_maps = [{"logit": x[b], "bits": bits[b]} for b in range(B)]
    res = bass_utils.run_bass_kernel_spmd(nc, in_maps, core_ids=list(range(B)))

    racc = 0.0
    wacc = 0.0
    for r in res.results:
        a = r["acc"]
        racc += float(a[:, :NCHUNK].sum(dtype=np.float64))
        wacc += float(a[:, NCHUNK:].sum(dtype=np.float64))

    if wacc <= MIN_KEPT * B:
        # quantile threshold exceeds 0.7 -> exact host path (rare/never for
        # the target distribution)
        return _host_fallback(seg_logit, seg_label)

    total = -(racc + C0 * wacc)
    return np.float32(total / N_TOTAL)



# revision 2
# speedup vs baseline: 1.0540x; 1.0540x over previous
"""OHEM cross-entropy loss kernel for Trainium2 (8 NeuronCores, Bass/Tile).

Math (matches reference.py):
    logp   = log_softmax(seg_logit, axis=1)          # [B,C,H,W], C=19
    x_l    = logp at label (ignore 255 -> class 0)
    prob   = exp(x_l)
    thr    = max(sort(prob.flatten())[MIN_KEPT*B], 0.7)
    loss   = mean(-x_l * (prob < thr))

Device strategy (data-parallel over B across 8 cores, one image per core):
    Per pixel: t = x_raw[label] - log(sum_c exp(x_raw[c])), u = t - log(0.7),
    w = 1[u < 0].  Device accumulates per-partition partials of
    sum(relu(-u)) (= -sum(min(u,0)) = -sum(u*w)) and sum(sign(u))
    (-> count of u<0); the host combines:
        loss = (sum_relu - log(0.7)*count) / N

    (valid when count > MIN_KEPT*B, which the host verifies; otherwise an
    exact host fallback computes the quantile path)

Per 128xF-pixel chunk on-chip:
      - one DMA loads [128, 19, F] f32 logits (class-major, 4B*F rows)
      - ACT: ONE Exp instruction over all 19*F elems -> bf16 eb
      - DVE: pairwise bulk adds (2x bf16) -> sumexp;  ACT: lse = Ln(sumexp)
      - DVE: label gather as a 5-instruction mux tree of strided
        copy_predicated merges keyed on broadcast label bit-planes
      - DVE: u = (x_l - log.7) - lse
      - ACT: Relu(-u) and Sign(u), each with accum_out -> [P, 1] partials

All four activation funcs (Exp, Ln, Relu, Sign) live in the single
'natural_log_exp_and_others' table set; get_activation_tables is patched
during finalize so the act-table pass picks that set once instead of
thrashing exp_and_others <-> natural_log every chunk.
"""

import numpy as np

B = 8
C = 19
H, W = 512, 1024
HW = H * W            # 524288 pixels per image/core
P = 128               # SBUF partitions
FREE = HW // P        # 4096 pixels per partition
F = 256               # chunk free size
NCHUNK = FREE // F    # 16
NBITS = 5             # ceil(log2(19))
C0 = float(np.log(np.float32(0.7)))
MIN_KEPT = 100000
IGNORE_INDEX = 255
N_TOTAL = B * HW

_CACHE = {}


def _build_nc():
    import concourse.bacc as bacc
    import concourse.mybir as mybir
    import concourse.tile as tile

    fp32 = mybir.dt.float32
    bf16 = mybir.dt.bfloat16
    u8 = mybir.dt.uint8
    AF = mybir.ActivationFunctionType

    nc = bacc.Bacc()
    logit = nc.dram_tensor("logit", [C, HW], fp32, kind="ExternalInput")
    bits = nc.dram_tensor("bits", [NBITS, P, FREE], u8, kind="ExternalInput")
    acc = nc.dram_tensor("acc", [P, 2 * NCHUNK], fp32, kind="ExternalOutput")

    # [C, (P FREE)] -> [P, C, FREE] view for chunked class-major loads
    logit_v = logit[:, :].rearrange("c (p f) -> p c f", p=P)

    with tile.TileContext(nc) as tc:
        with (
            tc.tile_pool(name="lb", bufs=3) as lb_pool,
            tc.tile_pool(name="eb", bufs=2) as eb_pool,
            tc.tile_pool(name="bits", bufs=1) as bits_pool,
            tc.tile_pool(name="pix", bufs=2) as pix_pool,
            tc.tile_pool(name="accp", bufs=1) as acc_pool,
        ):
            acc_t = acc_pool.tile([P, 2 * NCHUNK], fp32)
            bits_t = bits_pool.tile([P, NBITS, FREE], u8)
            # all 5 bit-planes in one DMA: [NBITS, P, FREE] -> [P, NBITS, FREE]
            nc.sync.dma_start(
                out=bits_t[:], in_=bits[:, :, :].rearrange("k p f -> p k f")
            )

            # deferred per-chunk tails so ACT's in-order queue never blocks
            # behind DVE: relu/sign of chunk j are traced after exp/ln of j+1
            pend = []

            def flush_tail():
                for u_, j_ in pend:
                    scr = pix_pool.tile([P, F], fp32, tag="scr")
                    # sum(relu(-u)) = -sum(min(u, 0)) = -sum(u * 1[u<0])
                    nc.scalar.activation(
                        out=scr[:], in_=u_[:], func=AF.Relu, scale=-1.0,
                        accum_out=acc_t[:, j_ : j_ + 1],
                    )
                    scr2 = pix_pool.tile([P, F], fp32, tag="scr2")
                    # sum(sign(u)) -> count(u<0) = (F*P*... - total)/2 on host
                    nc.scalar.activation(
                        out=scr2[:], in_=u_[:], func=AF.Sign,
                        accum_out=acc_t[:, NCHUNK + j_ : NCHUNK + j_ + 1],
                    )
                pend.clear()

            for j in range(NCHUNK):
                lb = lb_pool.tile([P, C, F], fp32, tag="lb")
                nc.sync.dma_start(out=lb[:], in_=logit_v[:, :, j * F : (j + 1) * F])

                # one Exp over the whole [P, 19*F] chunk, f32 -> bf16
                eb = eb_pool.tile([P, C, F], bf16, tag="eb")
                nc.scalar.activation(out=eb[:], in_=lb[:], func=AF.Exp)

                # drain previous chunk's reductions now that exp(j) is queued
                flush_tail()

                # sumexp: pairwise bulk adds in bf16 (2x mode)
                # tree: [0:9]+=[9:18]; [0:4]+=[4:8]; [8]+=[18]; [0:2]+=[2:4];
                #       [0]+=[1]; sumexp = [0]+[8] (bf16 out, stays 2x)
                nc.vector.tensor_tensor(
                    out=eb[:, 0:9, :], in0=eb[:, 0:9, :], in1=eb[:, 9:18, :],
                    op=mybir.AluOpType.add,
                )
                nc.vector.tensor_tensor(
                    out=eb[:, 0:4, :], in0=eb[:, 0:4, :], in1=eb[:, 4:8, :],
                    op=mybir.AluOpType.add,
                )
                nc.vector.tensor_tensor(
                    out=eb[:, 8, :], in0=eb[:, 8, :], in1=eb[:, 18, :],
                    op=mybir.AluOpType.add,
                )
                nc.vector.tensor_tensor(
                    out=eb[:, 0:2, :], in0=eb[:, 0:2, :], in1=eb[:, 2:4, :],
                    op=mybir.AluOpType.add,
                )
                nc.vector.tensor_tensor(
                    out=eb[:, 0, :], in0=eb[:, 0, :], in1=eb[:, 1, :],
                    op=mybir.AluOpType.add,
                )
                sumexp = pix_pool.tile([P, F], bf16, tag="sumexp")
                nc.vector.tensor_tensor(
                    out=sumexp[:], in0=eb[:, 0, :], in1=eb[:, 8, :],
                    op=mybir.AluOpType.add,
                )

                lse = pix_pool.tile([P, F], fp32, tag="lse")
                nc.scalar.activation(out=lse[:], in_=sumexp[:], func=AF.Ln)

                # label mux-tree gather, in place on lb (after exp read it);
                # each level is ONE strided copy_predicated with the bit-plane
                # mask broadcast across the merged slot pairs
                bs = bits_t[:, :, j * F : (j + 1) * F]  # [P, NBITS, F]

                def mask(k, n):
                    return bs[:, k, :].unsqueeze(1).broadcast_to([P, n, F])

                # L0 (bit 0): slots {0,2,..,16} <- {1,3,..,17}
                nc.vector.copy_predicated(
                    out=lb[:, 0:18:2, :], mask=mask(0, 9), data=lb[:, 1:19:2, :]
                )
                # L1 (bit 1): {0,4,8,12,16} <- {2,6,10,14,18}
                nc.vector.copy_predicated(
                    out=lb[:, 0:17:4, :], mask=mask(1, 5), data=lb[:, 2:19:4, :]
                )
                # L2 (bit 2): {0,8} <- {4,12}
                nc.vector.copy_predicated(
                    out=lb[:, 0:9:8, :], mask=mask(2, 2), data=lb[:, 4:13:8, :]
                )
                # L3 (bit 3): {0} <- {8}
                nc.vector.copy_predicated(
                    out=lb[:, 0, :], mask=bs[:, 3, :], data=lb[:, 8, :]
                )
                # L4 (bit 4): {0} <- {16}
                nc.vector.copy_predicated(
                    out=lb[:, 0, :], mask=bs[:, 4, :], data=lb[:, 16, :]
                )

                # u = (x_l - log0.7) - lse
                u = pix_pool.tile([P, F], fp32, tag="u")
                nc.vector.scalar_tensor_tensor(
                    out=u[:], in0=lb[:, 0, :], scalar=C0, in1=lse[:],
                    op0=mybir.AluOpType.subtract, op1=mybir.AluOpType.subtract,
                )
                pend.append((u, j))

            flush_tail()
            nc.sync.dma_start(out=acc[:, :], in_=acc_t[:])

    # Patch the act-table map so the insert_act_table_loads fixpoint picks
    # the one set containing ALL our funcs (Exp, Ln, Relu, Sign) instead of
    # thrashing exp_and_others <-> natural_log on every chunk. Indices of
    # the sets (= act_func_set_id) are preserved; only membership of the
    # non-target sets is masked.
    import concourse.bacc as bacc_mod
    import concourse.hw_specs as hw_mod

    AF = mybir.ActivationFunctionType
    target = "natural_log_exp_and_others"
    need = {AF.Exp, AF.Ln, AF.Relu, AF.Sign}
    orig = hw_mod.get_activation_tables

    def patched(arch):
        tabs = orig(arch)
        if target not in tabs or not need.issubset(tabs[target]):
            return tabs  # unexpected act_info; fall back to default behavior
        return {
            k: (v if k == target else {f for f in v if f not in need})
            for k, v in tabs.items()
        }

    bacc_mod.get_activation_tables = patched
    hw_mod.get_activation_tables = patched
    try:
        nc.finalize()  # Bacc: runs compile() (reg alloc, act-table pass, ...)
    finally:
        bacc_mod.get_activation_tables = orig
        hw_mod.get_activation_tables = orig
    return nc


def _host_fallback(seg_logit, seg_label):
    """Exact numpy replication of the reference (quantile path included)."""
    x = np.asarray(seg_logit, dtype=np.float32)
    lbl = np.asarray(seg_label)
    Bn, Cn = x.shape[0], x.shape[1]
    xf = x.reshape(Bn, Cn, -1)
    m = xf.max(axis=1, keepdims=True)
    e = np.exp(xf - m)
    lse = np.log(e.sum(axis=1, keepdims=True)) + m
    logp = xf - lse
    l2 = np.where(lbl == IGNORE_INDEX, 0, lbl).reshape(Bn, 1, -1).astype(np.int64)
    lp_at = np.take_along_axis(logp, l2, axis=1)[:, 0]
    prob = np.exp(lp_at)
    sortp = np.sort(prob.reshape(-1))
    idx = min(MIN_KEPT * Bn, sortp.shape[0] - 1)
    thr = max(float(sortp[idx]), np.float32(0.7))
    wgt = (prob < thr).astype(np.float32)
    return np.float32((-lp_at * wgt).mean())


def kernel(seg_logit, seg_label):
    from concourse import bass_utils

    x = np.ascontiguousarray(np.asarray(seg_logit, dtype=np.float32)).reshape(
        B, C, HW
    )
    lbl = np.asarray(seg_label)
    lbl = np.where(lbl == IGNORE_INDEX, 0, lbl).astype(np.uint8).reshape(B, P, FREE)
    # 5 bit-planes per core: [NBITS, P, FREE] uint8
    bits = np.stack(
        [((lbl >> k) & 1).astype(np.uint8) for k in range(NBITS)], axis=1
    )  # [B, NBITS, P, FREE]

    if "nc" not in _CACHE:
        _CACHE["nc"] = _build_nc()
    nc = _CACHE["nc"]

    in_maps = [{"logit": x[b], "bits": bits[b]} for b in range(B)]
    res = bass_utils.run_bass_kernel_spmd(nc, in_maps, core_ids=list(range(B)))

    relu_sum = 0.0
    sign_sum = 0.0
    for r in res.results:
        a = r["acc"]
        relu_sum += float(a[:, :NCHUNK].sum(dtype=np.float64))
        sign_sum += float(a[:, NCHUNK:].sum(dtype=np.float64))

    # count(u<0) from sum(sign(u)) (u==0 is measure-zero for this input)
    wacc = (N_TOTAL - sign_sum) / 2.0

    if wacc <= MIN_KEPT * B:
        # quantile threshold exceeds 0.7 -> exact host path (rare/never for
        # the target distribution)
        return _host_fallback(seg_logit, seg_label)

    # sum(-t*w) = sum(relu(-u)) - log(0.7)*count
    total = relu_sum - C0 * wacc
    return np.float32(total / N_TOTAL)


# revision 6
# speedup vs baseline: 1.1080x; 1.0512x over previous
"""OHEM cross-entropy loss kernel for Trainium2 (8 NeuronCores, Bass/Tile).

Math (matches reference.py):
    logp   = log_softmax(seg_logit, axis=1)          # [B,C,H,W], C=19
    x_l    = logp at label (ignore 255 -> class 0)
    prob   = exp(x_l)
    thr    = max(sort(prob.flatten())[MIN_KEPT*B], 0.7)
    loss   = mean(-x_l * (prob < thr))

Device strategy (data-parallel over B across 8 cores, one image per core):
    Per pixel: t = x_raw[label] - log(sum_c exp(x_raw[c])), u = t - log(0.7),
    w = 1[u < 0].  Device accumulates per-partition partials of
    sum(relu(-u)) (= -sum(min(u,0)) = -sum(u*w)) and sum(sign(u))
    (-> count of u<0); the host combines:
        loss = (sum_relu - log(0.7)*count) / N

    (valid when count > MIN_KEPT*B, which the host verifies; otherwise an
    exact host fallback computes the quantile path)

Per 128xF-pixel chunk on-chip:
      - one DMA loads [128, 19, F] f32 logits (class-major, 4B*F rows)
      - ACT: ONE Exp instruction over all 19*F elems -> bf16 eb
      - DVE: pairwise bulk adds (2x bf16) -> sumexp;  ACT: lse = Ln(sumexp)
      - DVE: label gather as a 5-instruction mux tree of strided
        copy_predicated merges keyed on broadcast label bit-planes
      - DVE: u = (x_l - log.7) - lse
      - ACT: Relu(-u) and Sign(u), each with accum_out -> [P, 1] partials

All four activation funcs (Exp, Ln, Relu, Sign) live in the single
'natural_log_exp_and_others' table set; get_activation_tables is patched
during finalize so the act-table pass picks that set once instead of
thrashing exp_and_others <-> natural_log every chunk.
"""

import numpy as np

B = 8
C = 19
H, W = 512, 1024
HW = H * W            # 524288 pixels per image/core
P = 128               # SBUF partitions
FREE = HW // P        # 4096 pixels per partition
# Variable chunk schedule: small chunks at the start (pipeline fill: DVE
# starts sooner) and end (tail: last chunk's post-DMA chain is short);
# big 512-pixel chunks in the middle keep DMA rows at 2KB (345 GB/s).
CHUNKS = [128, 128, 256, 512, 512, 512, 512, 512, 512, 256, 128, 128]
assert sum(CHUNKS) == FREE
NCHUNK = len(CHUNKS)  # 12
NBITS = 5             # ceil(log2(19))
C0 = float(np.log(np.float32(0.7)))
MIN_KEPT = 100000
IGNORE_INDEX = 255
N_TOTAL = B * HW

_CACHE = {}


def _build_nc():
    import concourse.bacc as bacc
    import concourse.mybir as mybir
    import concourse.tile as tile

    fp32 = mybir.dt.float32
    bf16 = mybir.dt.bfloat16
    u8 = mybir.dt.uint8
    AF = mybir.ActivationFunctionType

    nc = bacc.Bacc()
    logit = nc.dram_tensor("logit", [C, HW], fp32, kind="ExternalInput")
    bits = nc.dram_tensor("bits", [NBITS, P, FREE], u8, kind="ExternalInput")
    acc = nc.dram_tensor("acc", [P, 2 * NCHUNK], fp32, kind="ExternalOutput")

    # [C, (P FREE)] -> [P, C, FREE] view for chunked class-major loads
    logit_v = logit[:, :].rearrange("c (p f) -> p c f", p=P)

    with tile.TileContext(nc) as tc:
        with (
            tc.tile_pool(name="lb", bufs=3) as lb_pool,
            tc.tile_pool(name="eb", bufs=2) as eb_pool,
            tc.tile_pool(name="bits", bufs=1) as bits_pool,
            tc.tile_pool(name="pix", bufs=2) as pix_pool,
            tc.tile_pool(name="accp", bufs=1) as acc_pool,
        ):
            acc_t = acc_pool.tile([P, 2 * NCHUNK], fp32)
            bits_t = bits_pool.tile([P, NBITS, FREE], u8)
            # all 5 bit-planes in one DMA: [NBITS, P, FREE] -> [P, NBITS, FREE]
            nc.sync.dma_start(
                out=bits_t[:], in_=bits[:, :, :].rearrange("k p f -> p k f")
            )

            # deferred per-chunk tails so ACT's in-order queue never blocks
            # behind DVE: relu/sign of chunk j are traced after exp/ln of j+1
            pend = []

            def flush_tail():
                for u_, j_, fsz in pend:
                    scr = pix_pool.tile([P, fsz], fp32, tag="scr")
                    # sum(relu(-u)) = -sum(min(u, 0)) = -sum(u * 1[u<0])
                    nc.scalar.activation(
                        out=scr[:], in_=u_[:], func=AF.Relu, scale=-1.0,
                        accum_out=acc_t[:, j_ : j_ + 1],
                    )
                    scr2 = pix_pool.tile([P, fsz], fp32, tag="scr2")
                    # sum(sign(u)) -> count(u<0) = (N - total)/2 on host
                    nc.scalar.activation(
                        out=scr2[:], in_=u_[:], func=AF.Sign,
                        accum_out=acc_t[:, NCHUNK + j_ : NCHUNK + j_ + 1],
                    )
                pend.clear()

            off = 0
            for j, F in enumerate(CHUNKS):
                lb = lb_pool.tile([P, C, F], fp32, tag="lb")
                nc.sync.dma_start(out=lb[:], in_=logit_v[:, :, off : off + F])

                # one Exp over the whole [P, 19*F] chunk, f32 -> bf16
                eb = eb_pool.tile([P, C, F], bf16, tag="eb")
                nc.scalar.activation(out=eb[:], in_=lb[:], func=AF.Exp)

                # drain previous chunk's reductions now that exp(j) is queued
                flush_tail()

                # sumexp: pairwise bulk adds in bf16 (2x mode)
                # tree: [0:9]+=[9:18]; [0:4]+=[4:8]; [8]+=[18]; [0:2]+=[2:4];
                #       [0]+=[1]; sumexp = [0]+[8] (bf16 out, stays 2x)
                nc.vector.tensor_tensor(
                    out=eb[:, 0:9, :], in0=eb[:, 0:9, :], in1=eb[:, 9:18, :],
                    op=mybir.AluOpType.add,
                )
                nc.vector.tensor_tensor(
                    out=eb[:, 0:4, :], in0=eb[:, 0:4, :], in1=eb[:, 4:8, :],
                    op=mybir.AluOpType.add,
                )
                nc.vector.tensor_tensor(
                    out=eb[:, 8, :], in0=eb[:, 8, :], in1=eb[:, 18, :],
                    op=mybir.AluOpType.add,
                )
                nc.vector.tensor_tensor(
                    out=eb[:, 0:2, :], in0=eb[:, 0:2, :], in1=eb[:, 2:4, :],
                    op=mybir.AluOpType.add,
                )
                nc.vector.tensor_tensor(
                    out=eb[:, 0, :], in0=eb[:, 0, :], in1=eb[:, 1, :],
                    op=mybir.AluOpType.add,
                )
                sumexp = pix_pool.tile([P, F], bf16, tag="sumexp")
                nc.vector.tensor_tensor(
                    out=sumexp[:], in0=eb[:, 0, :], in1=eb[:, 8, :],
                    op=mybir.AluOpType.add,
                )

                lse = pix_pool.tile([P, F], fp32, tag="lse")
                nc.scalar.activation(out=lse[:], in_=sumexp[:], func=AF.Ln)

                # label mux-tree gather, in place on lb (after exp read it);
                # each level is ONE strided copy_predicated with the bit-plane
                # mask broadcast across the merged slot pairs
                bs = bits_t[:, :, off : off + F]  # [P, NBITS, F]

                def mask(k, n):
                    return bs[:, k, :].unsqueeze(1).broadcast_to([P, n, F])

                # L0 (bit 0): slots {0,2,..,16} <- {1,3,..,17}
                nc.vector.copy_predicated(
                    out=lb[:, 0:18:2, :], mask=mask(0, 9), data=lb[:, 1:19:2, :]
                )
                # L1 (bit 1): {0,4,8,12,16} <- {2,6,10,14,18}
                nc.vector.copy_predicated(
                    out=lb[:, 0:17:4, :], mask=mask(1, 5), data=lb[:, 2:19:4, :]
                )
                # L2 (bit 2): {0,8} <- {4,12}
                nc.vector.copy_predicated(
                    out=lb[:, 0:9:8, :], mask=mask(2, 2), data=lb[:, 4:13:8, :]
                )
                # L3 (bit 3): {0} <- {8}
                nc.vector.copy_predicated(
                    out=lb[:, 0, :], mask=bs[:, 3, :], data=lb[:, 8, :]
                )
                # L4 (bit 4): {0} <- {16}
                nc.vector.copy_predicated(
                    out=lb[:, 0, :], mask=bs[:, 4, :], data=lb[:, 16, :]
                )

                # u = (x_l - log0.7) - lse
                u = pix_pool.tile([P, F], fp32, tag="u")
                nc.vector.scalar_tensor_tensor(
                    out=u[:], in0=lb[:, 0, :], scalar=C0, in1=lse[:],
                    op0=mybir.AluOpType.subtract, op1=mybir.AluOpType.subtract,
                )
                pend.append((u, j, F))
                off += F

            flush_tail()
            nc.sync.dma_start(out=acc[:, :], in_=acc_t[:])

    # Patch the act-table map so the insert_act_table_loads fixpoint picks
    # the one set containing ALL our funcs (Exp, Ln, Relu, Sign) instead of
    # thrashing exp_and_others <-> natural_log on every chunk. Indices of
    # the sets (= act_func_set_id) are preserved; only membership of the
    # non-target sets is masked.
    import concourse.bacc as bacc_mod
    import concourse.hw_specs as hw_mod

    AF = mybir.ActivationFunctionType
    target = "natural_log_exp_and_others"
    need = {AF.Exp, AF.Ln, AF.Relu, AF.Sign}
    orig = hw_mod.get_activation_tables

    def patched(arch):
        tabs = orig(arch)
        if target not in tabs or not need.issubset(tabs[target]):
            return tabs  # unexpected act_info; fall back to default behavior
        return {
            k: (v if k == target else {f for f in v if f not in need})
            for k, v in tabs.items()
        }

    bacc_mod.get_activation_tables = patched
    hw_mod.get_activation_tables = patched
    try:
        nc.finalize()  # Bacc: runs compile() (reg alloc, act-table pass, ...)
    finally:
        bacc_mod.get_activation_tables = orig
        hw_mod.get_activation_tables = orig
    return nc


def _host_fallback(seg_logit, seg_label):
    """Exact numpy replication of the reference (quantile path included)."""
    x = np.asarray(seg_logit, dtype=np.float32)
    lbl = np.asarray(seg_label)
    Bn, Cn = x.shape[0], x.shape[1]
    xf = x.reshape(Bn, Cn, -1)
    m = xf.max(axis=1, keepdims=True)
    e = np.exp(xf - m)
    lse = np.log(e.sum(axis=1, keepdims=True)) + m
    logp = xf - lse
    l2 = np.where(lbl == IGNORE_INDEX, 0, lbl).reshape(Bn, 1, -1).astype(np.int64)
    lp_at = np.take_along_axis(logp, l2, axis=1)[:, 0]
    prob = np.exp(lp_at)
    sortp = np.sort(prob.reshape(-1))
    idx = min(MIN_KEPT * Bn, sortp.shape[0] - 1)
    thr = max(float(sortp[idx]), np.float32(0.7))
    wgt = (prob < thr).astype(np.float32)
    return np.float32((-lp_at * wgt).mean())


def kernel(seg_logit, seg_label):
    from concourse import bass_utils

    x = np.ascontiguousarray(np.asarray(seg_logit, dtype=np.float32)).reshape(
        B, C, HW
    )
    lbl = np.asarray(seg_label)
    lbl = np.where(lbl == IGNORE_INDEX, 0, lbl).astype(np.uint8).reshape(B, P, FREE)
    # 5 bit-planes per core: [NBITS, P, FREE] uint8
    bits = np.stack(
        [((lbl >> k) & 1).astype(np.uint8) for k in range(NBITS)], axis=1
    )  # [B, NBITS, P, FREE]

    if "nc" not in _CACHE:
        _CACHE["nc"] = _build_nc()
    nc = _CACHE["nc"]

    in_maps = [{"logit": x[b], "bits": bits[b]} for b in range(B)]
    res = bass_utils.run_bass_kernel_spmd(nc, in_maps, core_ids=list(range(B)))

    relu_sum = 0.0
    sign_sum = 0.0
    for r in res.results:
        a = r["acc"]
        relu_sum += float(a[:, :NCHUNK].sum(dtype=np.float64))
        sign_sum += float(a[:, NCHUNK:].sum(dtype=np.float64))

    # count(u<0) from sum(sign(u)) (u==0 is measure-zero for this input)
    wacc = (N_TOTAL - sign_sum) / 2.0

    if wacc <= MIN_KEPT * B:
        # quantile threshold exceeds 0.7 -> exact host path (rare/never for
        # the target distribution)
        return _host_fallback(seg_logit, seg_label)

    # sum(-t*w) = sum(relu(-u)) - log(0.7)*count
    total = relu_sum - C0 * wacc
    return np.float32(total / N_TOTAL)


# revision 15
# speedup vs baseline: 1.1411x; 1.0298x over previous
"""OHEM cross-entropy loss kernel for Trainium2 (8 NeuronCores, Bass/Tile).

Math (matches reference.py):
    logp   = log_softmax(seg_logit, axis=1)          # [B,C,H,W], C=19
    x_l    = logp at label (ignore 255 -> class 0)
    prob   = exp(x_l)
    thr    = max(sort(prob.flatten())[MIN_KEPT*B], 0.7)
    loss   = mean(-x_l * (prob < thr))

Device strategy (data-parallel over B across 8 cores, one image per core):
    Per pixel: t = x_raw[label] - log(sum_c exp(x_raw[c])), u = t - log(0.7),
    w = 1[u < 0].  Device accumulates per-partition partials of
    sum(relu(-u)) (= -sum(min(u,0)) = -sum(u*w)) and sum(sign(u))
    (-> count of u<0); the host combines:
        loss = (sum_relu - log(0.7)*count) / N

    (valid when count > MIN_KEPT*B, which the host verifies; otherwise an
    exact host fallback computes the quantile path)

Per 128xF-pixel chunk on-chip:
      - one DMA loads [128, 19, F] f32 logits (class-major, 4B*F rows)
      - ACT: ONE Exp instruction over all 19*F elems -> bf16 eb
      - DVE: pairwise bulk adds (2x bf16) -> sumexp;  ACT: lse = Ln(sumexp)
      - DVE: label gather as a 5-instruction mux tree of strided
        copy_predicated merges keyed on broadcast label bit-planes
      - DVE: u = (x_l - log.7) - lse
      - ACT: Relu(-u) and Sign(u), each with accum_out -> [P, 1] partials

All four activation funcs (Exp, Ln, Relu, Sign) live in the single
'natural_log_exp_and_others' table set; get_activation_tables is patched
during finalize so the act-table pass picks that set once instead of
thrashing exp_and_others <-> natural_log every chunk.
"""

import numpy as np

B = 8
C = 19
H, W = 512, 1024
HW = H * W            # 524288 pixels per image/core
P = 128               # SBUF partitions
FREE = HW // P        # 4096 pixels per partition
# Variable chunk schedule: small chunks at the start (pipeline fill: DVE
# starts sooner) and end (tail: last chunk's post-DMA chain is short);
# big 512-pixel chunks in the middle keep DMA rows at 2KB (345 GB/s).
CHUNKS = [256, 512, 512, 512, 512, 512, 512, 512, 256]
assert sum(CHUNKS) == FREE
NCHUNK = len(CHUNKS)  # 9
NBITS = 5             # ceil(log2(19))
C0 = float(np.log(np.float32(0.7)))
MIN_KEPT = 100000
IGNORE_INDEX = 255
N_TOTAL = B * HW

_CACHE = {}


def _build_nc():
    import concourse.bacc as bacc
    import concourse.mybir as mybir
    import concourse.tile as tile

    fp32 = mybir.dt.float32
    bf16 = mybir.dt.bfloat16
    u8 = mybir.dt.uint8
    AF = mybir.ActivationFunctionType

    nc = bacc.Bacc()
    logit = nc.dram_tensor("logit", [C, HW], fp32, kind="ExternalInput")
    bits = nc.dram_tensor("bits", [NBITS, P, FREE], u8, kind="ExternalInput")
    acc = nc.dram_tensor("acc", [P, 2 * NCHUNK], fp32, kind="ExternalOutput")

    # [C, (P FREE)] -> [P, C, FREE] view for chunked class-major loads
    logit_v = logit[:, :].rearrange("c (p f) -> p c f", p=P)

    with tile.TileContext(nc) as tc:
        with (
            tc.tile_pool(name="lb", bufs=3) as lb_pool,
            tc.tile_pool(name="eb", bufs=2) as eb_pool,
            tc.tile_pool(name="bits", bufs=1) as bits_pool,
            tc.tile_pool(name="pix", bufs=2) as pix_pool,
            tc.tile_pool(name="accp", bufs=1) as acc_pool,
        ):
            acc_t = acc_pool.tile([P, 2 * NCHUNK], fp32)
            bits_t = bits_pool.tile([P, NBITS, FREE], u8)
            # all 5 bit-planes in one DMA: [NBITS, P, FREE] -> [P, NBITS, FREE]
            nc.sync.dma_start(
                out=bits_t[:], in_=bits[:, :, :].rearrange("k p f -> p k f")
            )

            # deferred per-chunk tails so ACT's in-order queue never blocks
            # behind DVE: relu/sign of chunk j are traced after exp/ln of j+1
            pend = []

            def flush_tail():
                for u_, j_, fsz in pend:
                    scr = pix_pool.tile([P, fsz], fp32, tag="scr")
                    # sum(relu(-u)) = -sum(min(u, 0)) = -sum(u * 1[u<0])
                    nc.scalar.activation(
                        out=scr[:], in_=u_[:], func=AF.Relu, scale=-1.0,
                        accum_out=acc_t[:, j_ : j_ + 1],
                    )
                    scr2 = pix_pool.tile([P, fsz], fp32, tag="scr2")
                    # sum(sign(u)) -> count(u<0) = (N - total)/2 on host
                    nc.scalar.activation(
                        out=scr2[:], in_=u_[:], func=AF.Sign,
                        accum_out=acc_t[:, NCHUNK + j_ : NCHUNK + j_ + 1],
                    )
                pend.clear()

            off = 0
            for j, F in enumerate(CHUNKS):
                lb = lb_pool.tile([P, C, F], fp32, tag="lb")
                nc.sync.dma_start(out=lb[:], in_=logit_v[:, :, off : off + F])

                # one Exp over the whole [P, 19*F] chunk, f32 -> bf16
                eb = eb_pool.tile([P, C, F], bf16, tag="eb")
                nc.scalar.activation(out=eb[:], in_=lb[:], func=AF.Exp)

                # drain previous chunk's reductions now that exp(j) is queued
                flush_tail()

                # sumexp tree level 1 ([0:9] += [9:18]; bf16 2x)
                nc.vector.tensor_tensor(
                    out=eb[:, 0:9, :], in0=eb[:, 0:9, :], in1=eb[:, 9:18, :],
                    op=mybir.AluOpType.add,
                )

                # label mux-tree gather, in place on lb (after exp read it);
                # each level is ONE strided copy_predicated with the bit-plane
                # mask broadcast across the merged slot pairs
                bs = bits_t[:, :, off : off + F]  # [P, NBITS, F]

                def mask(k, n):
                    return bs[:, k, :].unsqueeze(1).broadcast_to([P, n, F])

                # L0 (bit 0): slots {0,2,..,16} <- {1,3,..,17}
                nc.vector.copy_predicated(
                    out=lb[:, 0:18:2, :], mask=mask(0, 9), data=lb[:, 1:19:2, :]
                )
                # L1 (bit 1): {0,4,8,12,16} <- {2,6,10,14,18}
                nc.vector.copy_predicated(
                    out=lb[:, 0:17:4, :], mask=mask(1, 5), data=lb[:, 2:19:4, :]
                )
                # L2 (bit 2): {0,8} <- {4,12}
                nc.vector.copy_predicated(
                    out=lb[:, 0:9:8, :], mask=mask(2, 2), data=lb[:, 4:13:8, :]
                )
                # L3 (bit 3): {0} <- {8}
                nc.vector.copy_predicated(
                    out=lb[:, 0, :], mask=bs[:, 3, :], data=lb[:, 8, :]
                )
                # L4 (bit 4): {0} <- {16}
                nc.vector.copy_predicated(
                    out=lb[:, 0, :], mask=bs[:, 4, :], data=lb[:, 16, :]
                )

                # rest of the sumexp tree (bf16 2x adds; level 1 was the
                # accum-DMA above): [0:4]+=[4:8]; [8]+=[18]; [0:2]+=[2:4];
                # [0]+=[1]; sumexp = [0]+[8]
                nc.vector.tensor_tensor(
                    out=eb[:, 0:4, :], in0=eb[:, 0:4, :], in1=eb[:, 4:8, :],
                    op=mybir.AluOpType.add,
                )
                nc.vector.tensor_tensor(
                    out=eb[:, 8, :], in0=eb[:, 8, :], in1=eb[:, 18, :],
                    op=mybir.AluOpType.add,
                )
                nc.vector.tensor_tensor(
                    out=eb[:, 0:2, :], in0=eb[:, 0:2, :], in1=eb[:, 2:4, :],
                    op=mybir.AluOpType.add,
                )
                nc.vector.tensor_tensor(
                    out=eb[:, 0, :], in0=eb[:, 0, :], in1=eb[:, 1, :],
                    op=mybir.AluOpType.add,
                )
                sumexp = pix_pool.tile([P, F], bf16, tag="sumexp")
                nc.vector.tensor_tensor(
                    out=sumexp[:], in0=eb[:, 0, :], in1=eb[:, 8, :],
                    op=mybir.AluOpType.add,
                )

                lse = pix_pool.tile([P, F], fp32, tag="lse")
                nc.scalar.activation(out=lse[:], in_=sumexp[:], func=AF.Ln)

                # u = (x_l - log0.7) - lse
                u = pix_pool.tile([P, F], fp32, tag="u")
                nc.vector.scalar_tensor_tensor(
                    out=u[:], in0=lb[:, 0, :], scalar=C0, in1=lse[:],
                    op0=mybir.AluOpType.subtract, op1=mybir.AluOpType.subtract,
                )
                pend.append((u, j, F))
                off += F

            flush_tail()
            nc.sync.dma_start(out=acc[:, :], in_=acc_t[:])

    # Patch the act-table map so the insert_act_table_loads fixpoint picks
    # the one set containing ALL our funcs (Exp, Ln, Relu, Sign) instead of
    # thrashing exp_and_others <-> natural_log on every chunk. Indices of
    # the sets (= act_func_set_id) are preserved; only membership of the
    # non-target sets is masked.
    import concourse.bacc as bacc_mod
    import concourse.hw_specs as hw_mod

    AF = mybir.ActivationFunctionType
    target = "natural_log_exp_and_others"
    need = {AF.Exp, AF.Ln, AF.Relu, AF.Sign}
    orig = hw_mod.get_activation_tables

    def patched(arch):
        tabs = orig(arch)
        if target not in tabs or not need.issubset(tabs[target]):
            return tabs  # unexpected act_info; fall back to default behavior
        return {
            k: (v if k == target else {f for f in v if f not in need})
            for k, v in tabs.items()
        }

    bacc_mod.get_activation_tables = patched
    hw_mod.get_activation_tables = patched
    try:
        nc.finalize()  # Bacc: runs compile() (reg alloc, act-table pass, ...)
    finally:
        bacc_mod.get_activation_tables = orig
        hw_mod.get_activation_tables = orig
    return nc


def _host_fallback(seg_logit, seg_label):
    """Exact numpy replication of the reference (quantile path included)."""
    x = np.asarray(seg_logit, dtype=np.float32)
    lbl = np.asarray(seg_label)
    Bn, Cn = x.shape[0], x.shape[1]
    xf = x.reshape(Bn, Cn, -1)
    m = xf.max(axis=1, keepdims=True)
    e = np.exp(xf - m)
    lse = np.log(e.sum(axis=1, keepdims=True)) + m
    logp = xf - lse
    l2 = np.where(lbl == IGNORE_INDEX, 0, lbl).reshape(Bn, 1, -1).astype(np.int64)
    lp_at = np.take_along_axis(logp, l2, axis=1)[:, 0]
    prob = np.exp(lp_at)
    sortp = np.sort(prob.reshape(-1))
    idx = min(MIN_KEPT * Bn, sortp.shape[0] - 1)
    thr = max(float(sortp[idx]), np.float32(0.7))
    wgt = (prob < thr).astype(np.float32)
    return np.float32((-lp_at * wgt).mean())


def kernel(seg_logit, seg_label):
    from concourse import bass_utils

    x = np.ascontiguousarray(np.asarray(seg_logit, dtype=np.float32)).reshape(
        B, C, HW
    )
    lbl = np.asarray(seg_label)
    lbl = np.where(lbl == IGNORE_INDEX, 0, lbl).astype(np.uint8).reshape(B, P, FREE)
    # 5 bit-planes per core: [NBITS, P, FREE] uint8
    bits = np.stack(
        [((lbl >> k) & 1).astype(np.uint8) for k in range(NBITS)], axis=1
    )  # [B, NBITS, P, FREE]

    if "nc" not in _CACHE:
        _CACHE["nc"] = _build_nc()
    nc = _CACHE["nc"]

    in_maps = [{"logit": x[b], "bits": bits[b]} for b in range(B)]
    res = bass_utils.run_bass_kernel_spmd(nc, in_maps, core_ids=list(range(B)))

    relu_sum = 0.0
    sign_sum = 0.0
    for r in res.results:
        a = r["acc"]
        relu_sum += float(a[:, :NCHUNK].sum(dtype=np.float64))
        sign_sum += float(a[:, NCHUNK:].sum(dtype=np.float64))

    # count(u<0) from sum(sign(u)) (u==0 is measure-zero for this input)
    wacc = (N_TOTAL - sign_sum) / 2.0

    if wacc <= MIN_KEPT * B:
        # quantile threshold exceeds 0.7 -> exact host path (rare/never for
        # the target distribution)
        return _host_fallback(seg_logit, seg_label)

    # sum(-t*w) = sum(relu(-u)) - log(0.7)*count
    total = relu_sum - C0 * wacc
    return np.float32(total / N_TOTAL)


# revision 19
# speedup vs baseline: 1.1841x; 1.0377x over previous
"""OHEM cross-entropy loss kernel for Trainium2 (8 NeuronCores, Bass/Tile).

Math (matches reference.py):
    logp   = log_softmax(seg_logit, axis=1)          # [B,C,H,W], C=19
    x_l    = logp at label (ignore 255 -> class 0)
    prob   = exp(x_l)
    thr    = max(sort(prob.flatten())[MIN_KEPT*B], 0.7)
    loss   = mean(-x_l * (prob < thr))

Device strategy (data-parallel over B across 8 cores, one image per core):
    Per pixel: t = x_raw[label] - log(sum_c exp(x_raw[c])), u = t - log(0.7),
    w = 1[u < 0].  Device accumulates per-partition partials of
    sum(relu(-u)) (= -sum(min(u,0)) = -sum(u*w)) and sum(sign(u))
    (-> count of u<0); the host combines:
        loss = (sum_relu - log(0.7)*count) / N

    (valid when count > MIN_KEPT*B, which the host verifies; otherwise an
    exact host fallback computes the quantile path)

Per 128xF-pixel chunk on-chip:
      - one DMA loads [128, 19, F] f32 logits (class-major, 4B*F rows)
      - ACT: ONE Exp instruction over all 19*F elems -> bf16 eb
      - DVE: pairwise bulk adds (2x bf16) -> sumexp;  ACT: lse = Ln(sumexp)
      - DVE: label gather as a 5-instruction mux tree of strided
        copy_predicated merges keyed on broadcast label bit-planes
      - DVE: u = (x_l - log.7) - lse
      - ACT: Relu(-u) and Sign(u), each with accum_out -> [P, 1] partials

All four activation funcs (Exp, Ln, Relu, Sign) live in the single
'natural_log_exp_and_others' table set; get_activation_tables is patched
during finalize so the act-table pass picks that set once instead of
thrashing exp_and_others <-> natural_log every chunk.
"""

import numpy as np

B = 8
C = 19
H, W = 512, 1024
HW = H * W            # 524288 pixels per image/core
P = 128               # SBUF partitions
FREE = HW // P        # 4096 pixels per partition
# Variable chunk schedule: small chunks at the start (pipeline fill: DVE
# starts sooner) and end (tail: last chunk's post-DMA chain is short);
# big 512-pixel chunks in the middle keep DMA rows at 2KB (345 GB/s).
CHUNKS = [128, 256, 512, 512, 512, 512, 512, 512, 512, 128]
assert sum(CHUNKS) == FREE
NCHUNK = len(CHUNKS)  # 10
NBITS = 5             # ceil(log2(19))
C0 = float(np.log(np.float32(0.7)))
MIN_KEPT = 100000
IGNORE_INDEX = 255
N_TOTAL = B * HW

_CACHE = {}


def _build_nc():
    import concourse.bacc as bacc
    import concourse.mybir as mybir
    import concourse.tile as tile

    fp32 = mybir.dt.float32
    bf16 = mybir.dt.bfloat16
    u8 = mybir.dt.uint8
    AF = mybir.ActivationFunctionType

    nc = bacc.Bacc()
    logit = nc.dram_tensor("logit", [C, HW], fp32, kind="ExternalInput")
    bits = nc.dram_tensor("bits", [NBITS, P, FREE], u8, kind="ExternalInput")
    acc = nc.dram_tensor("acc", [P, 2 * NCHUNK], fp32, kind="ExternalOutput")

    # [C, (P FREE)] -> [P, C, FREE] view for chunked class-major loads
    logit_v = logit[:, :].rearrange("c (p f) -> p c f", p=P)

    with tile.TileContext(nc) as tc:
        with (
            tc.tile_pool(name="lb", bufs=3) as lb_pool,
            tc.tile_pool(name="eb", bufs=2) as eb_pool,
            tc.tile_pool(name="bits", bufs=3) as bits_pool,
            tc.tile_pool(name="pix", bufs=2) as pix_pool,
            tc.tile_pool(name="accp", bufs=1) as acc_pool,
        ):
            acc_t = acc_pool.tile([P, 2 * NCHUNK], fp32)

            # deferred per-chunk tails so ACT's in-order queue never blocks
            # behind DVE: relu/sign of chunk j are traced after exp/ln of j+1
            pend = []

            def flush_tail():
                for u_, j_, fsz in pend:
                    scr = pix_pool.tile([P, fsz], fp32, tag="scr")
                    # sum(relu(-u)) = -sum(min(u, 0)) = -sum(u * 1[u<0])
                    nc.scalar.activation(
                        out=scr[:], in_=u_[:], func=AF.Relu, scale=-1.0,
                        accum_out=acc_t[:, j_ : j_ + 1],
                    )
                    scr2 = pix_pool.tile([P, fsz], fp32, tag="scr2")
                    # sum(sign(u)) -> count(u<0) = (N - total)/2 on host
                    nc.scalar.activation(
                        out=scr2[:], in_=u_[:], func=AF.Sign,
                        accum_out=acc_t[:, NCHUNK + j_ : NCHUNK + j_ + 1],
                    )
                pend.clear()

            off = 0
            for j, F in enumerate(CHUNKS):
                lb = lb_pool.tile([P, C, F], fp32, tag="lb")
                nc.sync.dma_start(out=lb[:], in_=logit_v[:, :, off : off + F])
                # per-chunk bit-plane load right behind the logit chunk, so
                # the mux never waits on one big up-front bits transfer
                bits_c = bits_pool.tile([P, NBITS, F], u8, tag="bits")
                nc.sync.dma_start(
                    out=bits_c[:],
                    in_=bits[:, :, off : off + F].rearrange("k p f -> p k f"),
                )

                # one Exp over the whole [P, 19*F] chunk, f32 -> bf16
                eb = eb_pool.tile([P, C, F], bf16, tag="eb")
                nc.scalar.activation(out=eb[:], in_=lb[:], func=AF.Exp)

                # drain previous chunk's reductions now that exp(j) is queued
                flush_tail()

                # sumexp tree level 1 ([0:9] += [9:18]; bf16 2x)
                nc.vector.tensor_tensor(
                    out=eb[:, 0:9, :], in0=eb[:, 0:9, :], in1=eb[:, 9:18, :],
                    op=mybir.AluOpType.add,
                )

                # label mux-tree gather, in place on lb (after exp read it);
                # each level is ONE strided copy_predicated with the bit-plane
                # mask broadcast across the merged slot pairs
                bs = bits_c[:]  # [P, NBITS, F]

                def mask(k, n):
                    return bs[:, k, :].unsqueeze(1).broadcast_to([P, n, F])

                # L0 (bit 0): slots {0,2,..,16} <- {1,3,..,17}
                nc.vector.copy_predicated(
                    out=lb[:, 0:18:2, :], mask=mask(0, 9), data=lb[:, 1:19:2, :]
                )
                # L1 (bit 1): {0,4,8,12,16} <- {2,6,10,14,18}
                nc.vector.copy_predicated(
                    out=lb[:, 0:17:4, :], mask=mask(1, 5), data=lb[:, 2:19:4, :]
                )
                # L2 (bit 2): {0,8} <- {4,12}
                nc.vector.copy_predicated(
                    out=lb[:, 0:9:8, :], mask=mask(2, 2), data=lb[:, 4:13:8, :]
                )
                # L3 (bit 3): {0} <- {8}
                nc.vector.copy_predicated(
                    out=lb[:, 0, :], mask=bs[:, 3, :], data=lb[:, 8, :]
                )
                # L4 (bit 4): {0} <- {16}
                nc.vector.copy_predicated(
                    out=lb[:, 0, :], mask=bs[:, 4, :], data=lb[:, 16, :]
                )

                # rest of the sumexp tree (bf16 2x adds; level 1 was the
                # accum-DMA above): [0:4]+=[4:8]; [8]+=[18]; [0:2]+=[2:4];
                # [0]+=[1]; sumexp = [0]+[8]
                nc.vector.tensor_tensor(
                    out=eb[:, 0:4, :], in0=eb[:, 0:4, :], in1=eb[:, 4:8, :],
                    op=mybir.AluOpType.add,
                )
                nc.vector.tensor_tensor(
                    out=eb[:, 8, :], in0=eb[:, 8, :], in1=eb[:, 18, :],
                    op=mybir.AluOpType.add,
                )
                nc.vector.tensor_tensor(
                    out=eb[:, 0:2, :], in0=eb[:, 0:2, :], in1=eb[:, 2:4, :],
                    op=mybir.AluOpType.add,
                )
                nc.vector.tensor_tensor(
                    out=eb[:, 0, :], in0=eb[:, 0, :], in1=eb[:, 1, :],
                    op=mybir.AluOpType.add,
                )
                sumexp = pix_pool.tile([P, F], bf16, tag="sumexp")
                nc.vector.tensor_tensor(
                    out=sumexp[:], in0=eb[:, 0, :], in1=eb[:, 8, :],
                    op=mybir.AluOpType.add,
                )

                lse = pix_pool.tile([P, F], fp32, tag="lse")
                nc.scalar.activation(out=lse[:], in_=sumexp[:], func=AF.Ln)

                # u = (x_l - log0.7) - lse
                u = pix_pool.tile([P, F], fp32, tag="u")
                nc.vector.scalar_tensor_tensor(
                    out=u[:], in0=lb[:, 0, :], scalar=C0, in1=lse[:],
                    op0=mybir.AluOpType.subtract, op1=mybir.AluOpType.subtract,
                )
                pend.append((u, j, F))
                off += F

            flush_tail()
            nc.sync.dma_start(out=acc[:, :], in_=acc_t[:])

    # Patch the act-table map so the insert_act_table_loads fixpoint picks
    # the one set containing ALL our funcs (Exp, Ln, Relu, Sign) instead of
    # thrashing exp_and_others <-> natural_log on every chunk. Indices of
    # the sets (= act_func_set_id) are preserved; only membership of the
    # non-target sets is masked.
    import concourse.bacc as bacc_mod
    import concourse.hw_specs as hw_mod

    AF = mybir.ActivationFunctionType
    target = "natural_log_exp_and_others"
    need = {AF.Exp, AF.Ln, AF.Relu, AF.Sign}
    orig = hw_mod.get_activation_tables

    def patched(arch):
        tabs = orig(arch)
        if target not in tabs or not need.issubset(tabs[target]):
            return tabs  # unexpected act_info; fall back to default behavior
        return {
            k: (v if k == target else {f for f in v if f not in need})
            for k, v in tabs.items()
        }

    bacc_mod.get_activation_tables = patched
    hw_mod.get_activation_tables = patched
    try:
        nc.finalize()  # Bacc: runs compile() (reg alloc, act-table pass, ...)
    finally:
        bacc_mod.get_activation_tables = orig
        hw_mod.get_activation_tables = orig
    return nc


def _host_fallback(seg_logit, seg_label):
    """Exact numpy replication of the reference (quantile path included)."""
    x = np.asarray(seg_logit, dtype=np.float32)
    lbl = np.asarray(seg_label)
    Bn, Cn = x.shape[0], x.shape[1]
    xf = x.reshape(Bn, Cn, -1)
    m = xf.max(axis=1, keepdims=True)
    e = np.exp(xf - m)
    lse = np.log(e.sum(axis=1, keepdims=True)) + m
    logp = xf - lse
    l2 = np.where(lbl == IGNORE_INDEX, 0, lbl).reshape(Bn, 1, -1).astype(np.int64)
    lp_at = np.take_along_axis(logp, l2, axis=1)[:, 0]
    prob = np.exp(lp_at)
    sortp = np.sort(prob.reshape(-1))
    idx = min(MIN_KEPT * Bn, sortp.shape[0] - 1)
    thr = max(float(sortp[idx]), np.float32(0.7))
    wgt = (prob < thr).astype(np.float32)
    return np.float32((-lp_at * wgt).mean())


def kernel(seg_logit, seg_label):
    from concourse import bass_utils

    x = np.ascontiguousarray(np.asarray(seg_logit, dtype=np.float32)).reshape(
        B, C, HW
    )
    lbl = np.asarray(seg_label)
    lbl = np.where(lbl == IGNORE_INDEX, 0, lbl).astype(np.uint8).reshape(B, P, FREE)
    # 5 bit-planes per core: [NBITS, P, FREE] uint8
    bits = np.stack(
        [((lbl >> k) & 1).astype(np.uint8) for k in range(NBITS)], axis=1
    )  # [B, NBITS, P, FREE]

    if "nc" not in _CACHE:
        _CACHE["nc"] = _build_nc()
    nc = _CACHE["nc"]

    in_maps = [{"logit": x[b], "bits": bits[b]} for b in range(B)]
    res = bass_utils.run_bass_kernel_spmd(nc, in_maps, core_ids=list(range(B)))

    relu_sum = 0.0
    sign_sum = 0.0
    for r in res.results:
        a = r["acc"]
        relu_sum += float(a[:, :NCHUNK].sum(dtype=np.float64))
        sign_sum += float(a[:, NCHUNK:].sum(dtype=np.float64))

    # count(u<0) from sum(sign(u)) (u==0 is measure-zero for this input)
    wacc = (N_TOTAL - sign_sum) / 2.0

    if wacc <= MIN_KEPT * B:
        # quantile threshold exceeds 0.7 -> exact host path (rare/never for
        # the target distribution)
        return _host_fallback(seg_logit, seg_label)

    # sum(-t*w) = sum(relu(-u)) - log(0.7)*count
    total = relu_sum - C0 * wacc
    return np.float32(total / N_TOTAL)
